# revision 1
# baseline (speedup 1.0000x reference)
"""Trainium2 Bass kernel for nn_Concurrent_13623636263650 (gnn_message_passing).

Math (per batch sample, N=2000 nodes, C=64):
  u      = res / ||res||_row                  (N, C)  unit rows
  raw    = u @ u.T with zeroed diag = u@u.T - I   (symmetric)
  gmax   = max(raw flat incl diag zeros), gmin = min(...)
  rng    = gmax - gmin
  rowsum = (u@t - 1 - N*gmin)/rng,  t = sum_n u_n
  d      = rowsum^-1/2
  h      = d * origin;  q = u.T@h;  sv = sum_n h_n
  x_g1   = d * ((u@q - h) - gmin*sv) / rng
  out    = tanh(M @ Wflat + origin @ bpool).T,  M[n, d*16+i] = origin[n,d]*x_g1[n,i]

Only gmin/gmax need the full N^2 gram; it is produced tile-by-tile on the PE
(f32r), cast fp32->fp16 on ACT, and min/max-scanned on DVE at 2x rate over the
upper triangle only.  Everything else uses small factorized matmuls.

Sharding: batch 16 across 8 cores (2 samples per core), SPMD program.
"""

import numpy as np
from contextlib import ExitStack

import concourse.bass as bass
import concourse.bacc as bacc
import concourse.tile as tile
from concourse import mybir
from concourse import bass_isa
from concourse.masks import make_identity, make_upper_triangular

B, NN, C = 16, 2000, 64
F, O = 16, 32
NCORES = 8
SPC = B // NCORES          # samples per core

FP32 = mybir.dt.float32
F32R = mybir.dt.float32r
FP16 = mybir.dt.float16
AX = mybir.AxisListType
AL = mybir.AluOpType
AF = mybir.ActivationFunctionType


def ap_view(sl, dims):
    """AP over slice `sl` keeping its partition dim/offset, with explicit
    [stride, count] free dims (element units; stride 0 broadcasts)."""
    return bass.AP(tensor=sl.tensor, offset=sl.offset, ap=[sl.ap[0]] + list(dims))


def build_program(nc, n_nodes=NN, spc=SPC):
    P = 125
    NCH = n_nodes // P           # node chunks
    FREE = 500 if n_nodes % 500 == 0 else n_nodes
    NT = n_nodes // FREE         # gram free-dim tiles
    RATIO = FREE // P            # P-chunks per free tile
    NG = NCH // RATIO            # chunk groups (of RATIO chunks)
    assert P * NCH == n_nodes and FREE * NT == n_nodes and RATIO * NT == NCH

    res_d = nc.dram_tensor("res", [spc, n_nodes, C], FP32, kind="ExternalInput").ap()
    org_d = nc.dram_tensor("origin", [spc, n_nodes, F], FP32, kind="ExternalInput").ap()
    wp_d = nc.dram_tensor("wpool", [F, F, O], FP32, kind="ExternalInput").ap()
    bp_d = nc.dram_tensor("bpool", [F, O], FP32, kind="ExternalInput").ap()
    out_d = nc.dram_tensor("out", [spc, O, n_nodes], FP32, kind="ExternalOutput").ap()

    with tile.TileContext(nc) as tc, ExitStack() as ctx:
        consts = ctx.enter_context(tc.tile_pool(name="consts", bufs=1))
        big = ctx.enter_context(tc.tile_pool(name="big", bufs=1))
        scal = ctx.enter_context(tc.tile_pool(name="scal", bufs=1))

        # ---------------- Phase A: loads & constants ----------------
        u_nc = big.tile([P, spc, NCH, C], FP32)     # res, then normalized u
        nc.sync.dma_start(out=u_nc, in_=res_d.rearrange("s (i p) c -> p s i c", p=P))
        or2 = big.tile([P, spc, NCH, F], FP32)
        nc.sync.dma_start(out=or2, in_=org_d.rearrange("s (i p) c -> p s i c", p=P))
        w2 = consts.tile([128, 2, O], F32R)          # f-tile k rows: (d%8)*16+i
        nc.sync.dma_start(out=w2, in_=wp_d.rearrange("(k d) i o -> (d i) k o", k=2).bitcast(F32R))
        bp = consts.tile([F, O], F32R)
        nc.sync.dma_start(out=bp, in_=bp_d.bitcast(F32R))

        ident = consts.tile([P, P], FP32)
        make_identity(nc, ident)
        wedge = consts.tile([P, P], FP16)
        make_upper_triangular(nc, wedge, val=1.0, diag=False)

        # ---------------- Phase B: row norms + normalize (DVE) ----------------
        sq = big.tile([P, spc * NCH * C], FP32)      # scratch
        u_fl = u_nc.rearrange("p s i c -> p (s i c)")
        nc.vector.tensor_tensor(out=sq, in0=u_fl, in1=u_fl, op=AL.mult)
        nrm2 = scal.tile([P, spc, NCH], FP32)
        nc.vector.tensor_reduce(nrm2, sq.rearrange("p (s i c) -> p s i c", s=spc, i=NCH),
                                axis=AX.X, op=AL.add)
        rn = scal.tile([P, spc, NCH], FP32)
        nc.scalar.activation(out=rn, in_=nrm2, func=AF.Sqrt)
        nc.vector.reciprocal(out=rn, in_=rn)
        nc.vector.tensor_tensor(
            out=u_nc, in0=u_nc,
            in1=ap_view(rn, [[NCH, spc], [1, NCH], [0, C]]), op=AL.mult)

        # ---------------- Phase C: uT & origin_T via PE transposes ----------------
        uT = big.tile([64, spc, n_nodes], F32R)      # per-sample, base partition 0
        oT = big.tile([F, spc, n_nodes], F32R)
        with tc.tile_pool(name="pc", bufs=2, space="PSUM") as pc:
            for g in range(NG):
                tps = pc.tile([64, spc, 512], FP32, tag="tps")
                ops_ = pc.tile([F, spc, 512], FP32, tag="ops")
                for k in range(RATIO):
                    i = g * RATIO + k
                    for s in range(spc):
                        nc.tensor.transpose(
                            tps[:, s, k * P:(k + 1) * P],
                            u_nc[:, s, i, :], ident)
                        nc.tensor.transpose(
                            ops_[:, s, k * P:(k + 1) * P],
                            or2[:, s, i, :], ident)
                nc.scalar.copy(uT[:, :, g * FREE:(g + 1) * FREE], tps[:, :, 0:FREE])
                nc.scalar.copy(oT[:, :, g * FREE:(g + 1) * FREE], ops_[:, :, 0:FREE])

        # ---------------- Phase E: gram (f32r) + fp16 min/max scan ----------------
        gmax_r = scal.tile([P, spc], FP32)           # replicated per-sample scalars
        gmin_r = scal.tile([P, spc], FP32)           # holds -gmin after negate
        with tc.tile_pool(name="pg", bufs=2, space="PSUM") as pg, \
             tc.tile_pool(name="sg", bufs=3) as sg, \
             tc.tile_pool(name="sacc", bufs=2) as sacc:
            # Interleave the two samples' rows: each sample's accumulator
            # chain is serial on DVE, but the two chains are independent, so
            # alternating rows lets one sample's scan fill the other's
            # cast/matmul latency bubbles.
            accs = []
            for s in range(spc):
                acc_mx = sacc.tile([P, n_nodes], FP16, tag="amx")
                acc_mn = sacc.tile([P, n_nodes], FP16, tag="amn")
                nc.vector.memset(acc_mx, -2.0)
                nc.vector.memset(acc_mn, 2.0)
                accs.append((acc_mx, acc_mn))
            for idx in range(spc * NCH):
                s, i = idx % spc, idx // spc
                acc_mx, acc_mn = accs[s]
                js = i // RATIO
                d_off = P * (i % RATIO)
                wi = (FREE - d_off) + FREE * (NT - 1 - js)
                gps = pg.tile([P, NT, 512], FP32, tag="gps")
                for j in range(js, NT):
                    nc.tensor.matmul(
                        gps[:, j, 0:FREE],
                        uT[:, s, i * P:(i + 1) * P],
                        uT[:, s, j * FREE:(j + 1) * FREE],
                        start=True, stop=True)
                g16 = sg.tile([P, n_nodes], FP16, tag="g16")
                nc.scalar.activation(out=g16[:, 0:FREE - d_off],
                                     in_=gps[:, js, d_off:FREE], func=AF.Copy)
                nfull = NT - 1 - js
                if nfull > 0:
                    nc.scalar.activation(
                        out=g16[:, FREE - d_off:wi].rearrange(
                            "p (t w) -> p t w", w=FREE),
                        in_=gps[:, js + 1:NT, 0:FREE], func=AF.Copy)
                nc.vector.tensor_tensor(out=g16[:, 0:P], in0=g16[:, 0:P],
                                        in1=wedge, op=AL.mult)
                nc.vector.tensor_tensor(out=acc_mx[:, 0:wi], in0=acc_mx[:, 0:wi],
                                        in1=g16[:, 0:wi], op=AL.max)
                nc.vector.tensor_tensor(out=acc_mn[:, 0:wi], in0=acc_mn[:, 0:wi],
                                        in1=g16[:, 0:wi], op=AL.min)
            for s in range(spc):
                acc_mx, acc_mn = accs[s]
                mx_p = scal.tile([P, 1], FP32, tag="mxp")
                mn_p = scal.tile([P, 1], FP32, tag="mnp")
                nc.vector.tensor_reduce(mx_p, acc_mx, axis=AX.X, op=AL.max)
                nc.vector.tensor_reduce(mn_p, acc_mn, axis=AX.X, op=AL.min)
                nc.vector.tensor_scalar(out=mx_p, in0=mx_p, scalar1=0.0, scalar2=None,
                                        op0=AL.max)
                nc.vector.tensor_scalar(out=mn_p, in0=mn_p, scalar1=0.0, scalar2=-1.0,
                                        op0=AL.min, op1=AL.mult)
                nc.gpsimd.partition_all_reduce(gmax_r[:, s:s + 1], mx_p, channels=P,
                                               reduce_op=bass_isa.ReduceOp.max)
                nc.gpsimd.partition_all_reduce(gmin_r[:, s:s + 1], mn_p, channels=P,
                                               reduce_op=bass_isa.ReduceOp.max)

        inv_r = scal.tile([P, spc], FP32)            # 1/rng; note gmin_r = -gmin
        nc.vector.tensor_tensor(out=inv_r, in0=gmax_r, in1=gmin_r, op=AL.add)
        nc.vector.reciprocal(out=inv_r, in_=inv_r)
        neg_gmin = gmin_r                            # alias for clarity

        # ---------------- Phase F: t, rowsum, d ----------------
        t2 = scal.tile([64, spc], FP32)
        tsc = big.tile([64, spc, n_nodes], FP32)     # throwaway activation out
        for s in range(spc):
            nc.scalar.activation(out=tsc[:, s], in_=uT[:, s], func=AF.Copy,
                                 accum_out=t2[:, s:s + 1])
        d2 = scal.tile([P, spc, NCH], FP32)
        with tc.tile_pool(name="pf", bufs=1, space="PSUM") as pf:
            rs_ps = pf.tile([P, spc, NCH], FP32)
            for s in range(spc):
                for i in range(NCH):
                    nc.tensor.matmul(rs_ps[:, s, i:i + 1],
                                     uT[:, s, i * P:(i + 1) * P].bitcast(FP32),
                                     t2[:, s:s + 1], start=True, stop=True)
            # rowsum_norm = (rs - 1 + N*neg_gmin) * inv;  d = 1/sqrt(rowsum_norm)
            bv = scal.tile([P, spc], FP32)
            nc.vector.tensor_scalar(out=bv, in0=neg_gmin, scalar1=float(n_nodes),
                                    scalar2=-1.0, op0=AL.mult, op1=AL.add)
            nc.vector.tensor_tensor(out=bv, in0=bv, in1=inv_r, op=AL.mult)
            for s in range(spc):
                nc.scalar.activation(out=d2[:, s, :], in_=rs_ps[:, s, :], func=AF.Sqrt,
                                     scale=inv_r[:, s:s + 1], bias=bv[:, s:s + 1])
        nc.vector.reciprocal(out=d2, in_=d2)

        # ---------------- Phase G: h, q, sv ----------------
        h2 = big.tile([P, spc, NCH, F], FP32)
        nc.vector.tensor_tensor(out=h2, in0=or2,
                                in1=ap_view(d2, [[NCH, spc], [1, NCH], [0, F]]),
                                op=AL.mult)
        q_sb = scal.tile([64, spc, F], F32R)
        with tc.tile_pool(name="pq", bufs=1, space="PSUM") as pq:
            q_ps = pq.tile([64, spc, F], FP32)
            for s in range(spc):
                for i in range(NCH):
                    nc.tensor.matmul(q_ps[:, s, :], u_nc[:, s, i, :],
                                     h2[:, s, i, :], start=(i == 0), stop=(i == NCH - 1))
            nc.scalar.copy(q_sb, q_ps)
        sv_t = scal.tile([P, spc, F], FP32)
        nc.vector.tensor_reduce(
            sv_t, ap_view(h2, [[NCH * F, spc], [1, F], [F, NCH]]), axis=AX.X, op=AL.add)
        sv_r = scal.tile([P, spc, F], FP32)
        nc.gpsimd.partition_all_reduce(
            sv_r.rearrange("p s f -> p (s f)"), sv_t.rearrange("p s f -> p (s f)"),
            channels=P, reduce_op=bass_isa.ReduceOp.add)

        # ---------------- Phase H: v, x_g1 ----------------
        xg1 = big.tile([P, spc, NCH, F], FP32)
        with tc.tile_pool(name="pv", bufs=2, space="PSUM") as pv:
            for s in range(spc):
                v_ps = pv.tile([P, NCH, F], FP32, tag="vps")
                for i in range(NCH):
                    nc.tensor.matmul(v_ps[:, i, :],
                                     uT[:, s, i * P:(i + 1) * P],
                                     q_sb[:, s, :], start=True, stop=True)
                gsv = scal.tile([P, F], FP32, tag="gsv")   # gmin*sv = -(neg_gmin*sv)
                nc.vector.tensor_scalar(out=gsv, in0=sv_r[:, s, :],
                                        scalar1=neg_gmin[:, s:s + 1], scalar2=None,
                                        op0=AL.mult)
                # xg1 = (v - h) + neg_gmin*sv
                nc.vector.tensor_tensor(out=xg1[:, s], in0=v_ps, in1=h2[:, s],
                                        op=AL.subtract)
                nc.vector.tensor_tensor(out=xg1[:, s], in0=xg1[:, s],
                                        in1=ap_view(gsv, [[0, NCH], [1, F]]),
                                        op=AL.add)
                dsc = scal.tile([P, NCH], FP32, tag="dsc")
                nc.vector.tensor_scalar(out=dsc, in0=d2[:, s, :],
                                        scalar1=inv_r[:, s:s + 1], scalar2=None,
                                        op0=AL.mult)
                nc.vector.tensor_tensor(out=xg1[:, s], in0=xg1[:, s],
                                        in1=ap_view(dsc, [[1, NCH], [0, F]]),
                                        op=AL.mult)

        # ---------------- Phase I+J: M build, transpose, final matmuls ----------------
        with tc.tile_pool(name="pm", bufs=2, space="PSUM") as pm, \
             tc.tile_pool(name="po", bufs=2, space="PSUM") as po, \
             tc.tile_pool(name="sm", bufs=2) as sm:
            for s in range(spc):
                mt0 = sm.tile([128, n_nodes], F32R, tag="mt0")
                mt1 = sm.tile([128, n_nodes], F32R, tag="mt1")
                for g in range(NG):
                    mg = sm.tile([P, RATIO, F, F], FP32, tag="mg")
                    # M[p, k, d, i] = origin[p, s, g*R+k, d] * xg1[p, s, g*R+k, i]
                    nc.gpsimd.tensor_tensor(
                        out=mg,
                        in0=ap_view(or2[:, s, g * RATIO:(g + 1) * RATIO, :],
                                    [[F, RATIO], [1, F], [0, F]]),
                        in1=ap_view(xg1[:, s, g * RATIO:(g + 1) * RATIO, :],
                                    [[F, RATIO], [0, F], [1, F]]),
                        op=AL.mult)
                    mtp0 = pm.tile([128, 512], FP32, tag="mtp0")
                    mtp1 = pm.tile([128, 512], FP32, tag="mtp1")
                    for k in range(RATIO):
                        mg_f = mg[:, k].rearrange("p d i -> p (d i)")
                        nc.tensor.transpose(mtp0[:, k * P:(k + 1) * P],
                                            mg_f[:, 0:128], ident)
                        nc.tensor.transpose(mtp1[:, k * P:(k + 1) * P],
                                            mg_f[:, 128:256], ident)
                    nc.scalar.copy(mt0[:, g * FREE:(g + 1) * FREE], mtp0[:, 0:FREE])
                    nc.vector.tensor_copy(mt1[:, g * FREE:(g + 1) * FREE],
                                          mtp1[:, 0:FREE])
                for k in range(NT):
                    ob = po.tile([O, FREE], FP32, tag="ob")
                    sl = slice(k * FREE, (k + 1) * FREE)
                    nc.tensor.matmul(ob, w2[:, 0, :],
                                     mt0[:, sl], start=True, stop=False)
                    nc.tensor.matmul(ob, w2[:, 1, :],
                                     mt1[:, sl], start=False, stop=False)
                    nc.tensor.matmul(ob, bp,
                                     oT[:, s, sl], start=False, stop=True)
                    osb = sm.tile([O, FREE], FP32, tag="osb")
                    nc.scalar.activation(out=osb, in_=ob, func=AF.Tanh)
                    nc.sync.dma_start(out=out_d[s, :, sl], in_=osb)
    return nc


_PROGRAM = None


def _get_program():
    global _PROGRAM
    if _PROGRAM is None:
        nc = bacc.Bacc("TRN2", target_bir_lowering=False, debug=False,
                       num_devices=NCORES)
        build_program(nc)
        nc.compile()
        _PROGRAM = nc
    return _PROGRAM


def kernel(**inputs):
    from concourse.bass_utils import run_bass_kernel_spmd
    res = np.asarray(inputs["res_x"], dtype=np.float32)
    org = np.asarray(inputs["origin_x"], dtype=np.float32)
    wp = np.asarray(inputs["weights_pool_x"], dtype=np.float32)
    bpl = np.asarray(inputs["bias_pool_x"], dtype=np.float32)
    nc = _get_program()
    in_maps = [
        {"res": res[c * SPC:(c + 1) * SPC], "origin": org[c * SPC:(c + 1) * SPC],
         "wpool": wp, "bpool": bpl}
        for c in range(NCORES)
    ]
    r = run_bass_kernel_spmd(nc, in_maps, list(range(NCORES)))
    out = np.concatenate([r.results[c]["out"] for c in range(NCORES)], axis=0)
    return out.astype(np.float32)

